# revision 1
# baseline (speedup 1.0000x reference)
"""CRF negative-log-likelihood kernel for Trainium2 (8 NeuronCores).

Math: reference computes  partition - gold  where
  partition = sum_b logsumexp_c(alpha[511])  via the forward algorithm
  gold      = sum emissions[b,s,tags] * m + sum T[tags[s],tags[s+1]] * m[:,1:]

Device strategy (data-parallel over batch, 32 rows per core):
  * Linear domain: alpha_t = E_t o (A @ alpha_{t-1}) with A = exp(T),
    E_t = exp(e_t).  The per-step logsumexp becomes a [128,128]x[128,32]
    matmul (PE) plus an elementwise multiply (DVE).
  * Bidirectional scan, PAIRED: forward (A) and backward (A^T) states
    live in one [128,64] tile [alphaF | vB]; the host lays emissions out
    so pair-step k holds [E_k | E_{511-k}].  Per step: 2 matmuls into one
    PSUM tile + ONE DVE multiply -> one semaphore round-trip per step.
    256 serial steps instead of 511 (the chain is latency-bound).
  * Stability: every RENORM steps rescale columns by 1/colsum (colsum via
    ones-matmul, reciprocal on DVE, broadcast via a tiny second matmul)
    applied RDELAY steps later by pre-scaling that E tile on GPSIMD.
    Raw column sums stream to the host, which adds sum(log(cs)) in f64.
  * Gold emit: masked sum eT o onehot(tags) chunk-wise: multiply on
    GPSIMD, free-axis sum via ScalarE activation accum_out.
  * Gold trans: exact pair-count matrix CNT[c,c'] accumulated on PE from
    host-built one-hot tiles (128 matmuls into one PSUM bank), then
    mul+reduce against T on DVE.  All gold work is INJECTED into the scan
    loop at controlled points so big Pool/ACT ops never sit ahead of
    renorm work in the strict per-engine FIFOs.
Outputs per core: colsum rows, meeting-dot row, gold partials; host sums
in float64 and returns a float32 scalar.
"""

import sys

for _p in ("/opt/trn_rl_repo",):
    if _p not in sys.path:
        sys.path.insert(0, _p)

import os as _os
import numpy as np
import ml_dtypes
from contextlib import ExitStack

from concourse import bass, tile, mybir, bacc
from concourse.bass_utils import run_bass_kernel_spmd

NCORES = 8
B, S, C = 256, 512, 128
BC = B // NCORES          # batch rows per core
FREE = S * BC             # free-dim elements of the per-core emission tensor
PAIRW = 2 * BC            # 64: [E_k | E_{S-1-k}]
RENORM = 6                # rescale period (pair-steps)
RDELAY = 5                # rescale applied this many steps after computed
HALF = S // 2             # pair-steps: fwd e_0..e_255, bwd e_256..e_511
NPAIR = BC * (S - 1)      # transition pairs per core (16352)

# emission chunk sizes (free elements); small leading chunks let the scan
# chain start before the bulk DMA+exp completes
CH_SIZES = [256, 768, 1024] + [2048] * 7
CH_OFF = [0]
for _s in CH_SIZES:
    CH_OFF.append(CH_OFF[-1] + _s)
assert CH_OFF[-1] == FREE
NCHUNK = len(CH_SIZES)

F32 = mybir.dt.float32
BF16 = mybir.dt.bfloat16
AF = mybir.ActivationFunctionType
OP = mybir.AluOpType

_EN_GOLD = _os.environ.get("CRF_GOLD", "1") == "1"
_EN_EMIT = _os.environ.get("CRF_EMIT", "1") == "1"
_EN_TRANS = _os.environ.get("CRF_TRANS", "1") == "1"
_EN_SCAN = _os.environ.get("CRF_SCAN", "1") == "1"

_NC_CACHE = None


def _build_nc():
    nc = bacc.Bacc("TRN2", target_bir_lowering=False, debug=False)

    NREN = len([k for k in range(1, HALF)
                if k % RENORM == 0 and k + RDELAY < HALF]) + 1

    et = nc.dram_tensor("et", [C, FREE], BF16, kind="ExternalInput").ap()
    afwd = nc.dram_tensor("afwd", [C, C], BF16, kind="ExternalInput").ap()
    abwd = nc.dram_tensor("abwd", [C, C], BF16, kind="ExternalInput").ap()
    hemit = nc.dram_tensor("hemit", [C, FREE], BF16, kind="ExternalInput").ap()
    cnt_in = nc.dram_tensor("cnt", [C, C], F32, kind="ExternalInput").ap()
    tsb_in = nc.dram_tensor("tsb", [C, C], F32, kind="ExternalInput").ap()
    cspair = nc.dram_tensor("cspair", [1, NREN * PAIRW], F32,
                            kind="ExternalOutput").ap()
    pdrow = nc.dram_tensor("pdrow", [1, BC], F32, kind="ExternalOutput").ap()
    gold = nc.dram_tensor("gold", [128, 1], F32, kind="ExternalOutput").ap()

    with tile.TileContext(nc) as tc, ExitStack() as ctx:
        sb = ctx.enter_context(tc.tile_pool(name="sb", bufs=1))
        wk = ctx.enter_context(tc.tile_pool(name="wk", bufs=4))
        ps = ctx.enter_context(tc.tile_pool(name="ps", bufs=2, space="PSUM"))

        # ---- persistent tiles -------------------------------------------
        wf = sb.tile([C, C], BF16, name="wf")
        wb_ = sb.tile([C, C], BF16, name="wb")
        nc.sync.dma_start(wf[:], afwd[:])
        nc.sync.dma_start(wb_[:], abwd[:])

        ones_col = sb.tile([C, 1], BF16, name="ones_col")
        ones_row = sb.tile([1, C], BF16, name="ones_row")
        nc.vector.memset(ones_col[:], 1.0)
        nc.vector.memset(ones_row[:], 1.0)

        cspair_sb = sb.tile([1, NREN * PAIRW], F32, name="cspair_sb")

        # ---- emission chunks: DMA in + exp ------------------------------
        raws, ecs = [], []
        et_dmas = []
        for k, csz in enumerate(CH_SIZES):
            raw = sb.tile([C, csz], BF16, name=f"raw{k}")
            et_dmas.append(
                nc.sync.dma_start(raw[:], et[:, CH_OFF[k]:CH_OFF[k] + csz]))
            raws.append(raw)
            ec = sb.tile([C, csz], BF16, name=f"ec{k}")
            ecs.append(ec)

        NEARLY = 2            # chunks whose exp runs before the scan starts
        def exp_chunk(c):
            nc.scalar.activation(ecs[c][:], raws[c][:], AF.Exp)
        for c in range(NEARLY):
            exp_chunk(c)

        def ec_pair(k):
            pos = k * PAIRW
            for c in range(NCHUNK):
                if pos < CH_OFF[c + 1]:
                    o = pos - CH_OFF[c]
                    return ecs[c][:, o:o + PAIRW]
            raise IndexError(k)

        # ---- gold inputs + injectable compute bodies --------------------
        if not _EN_GOLD:
            zg = sb.tile([128, 1], F32, name="zg")
            nc.vector.memset(zg[:], 0.0)
            nc.sync.dma_start(gold[:], zg[:])
        if not _EN_SCAN:
            zl = sb.tile([1, BC], F32, name="zl")
            nc.vector.memset(zl[:], 0.0)
            nc.sync.dma_start(pdrow[:], zl[:])
            zcf = sb.tile([1, NREN * PAIRW], F32, name="zcf")
            nc.vector.memset(zcf[:], 1.0)
            nc.sync.dma_start(cspair[:], zcf[:])

        from concourse.tile_rust import add_dep_helper
        gold_finish = None
        if _EN_GOLD:
            hem_sb = sb.tile([C, FREE], BF16, name="hem_sb")
            cnt_sb = sb.tile([C, C], F32, name="cnt_sb")
            tsb = sb.tile([C, C], F32, name="tsb_t")
            last_et = et_dmas[-1].ins
            qs = FREE // 8
            for k in range(8):
                gd = nc.sync.dma_start(hem_sb[:, k * qs:(k + 1) * qs],
                                       hemit[:, k * qs:(k + 1) * qs])
                add_dep_helper(gd.ins, last_et,
                               reason="gold DMA after emission stream")
            for gd in (nc.sync.dma_start(cnt_sb[:], cnt_in[:]),
                       nc.sync.dma_start(tsb[:], tsb_in[:])):
                add_dep_helper(gd.ins, last_et,
                               reason="gold DMA after emission stream")

            gold_acc = sb.tile([128, 1], F32, name="gold_acc")
            nc.vector.memset(gold_acc[:], 0.0)

            # emit work split into <=512-wide pieces, each anchored to a
            # scan step so Pool/ACT bursts stay inside one renorm window
            pieces = []
            for c, csz in enumerate(CH_SIZES):
                o = 0
                while o < csz:
                    w = min(512, csz - o)
                    pieces.append((c, o, w))
                    o += w

            def emit_piece(j, anchor):
                c, o, w = pieces[j]
                scratch = wk.tile([C, 512], BF16, tag="scr", bufs=2,
                                  name=f"scr{j}")
                epk = wk.tile([128, 1], F32, tag="ep", bufs=4, name=f"ep{j}")
                pool_inst = nc.gpsimd.tensor_mul(
                    scratch[:, 0:w], raws[c][:, o:o + w],
                    hem_sb[:, CH_OFF[c] + o:CH_OFF[c] + o + w])
                if anchor is not None:
                    add_dep_helper(pool_inst.ins, anchor.ins,
                                   reason="emit piece anchored to scan step")
                nc.scalar.activation(scratch[:, 0:w], scratch[:, 0:w],
                                     AF.Identity, accum_out=epk[:])
                nc.vector.tensor_add(gold_acc[:], gold_acc[:], epk[:])

            def gold_finish():
                gold_sb = sb.tile([128, 1], F32, name="gold_sb")
                nc.vector.tensor_copy(gold_sb[:], gold_acc[:])
                if _EN_TRANS:
                    trash = sb.tile([128, 128], F32, name="trash")
                    tp = sb.tile([128, 1], F32, name="tp")
                    nc.vector.tensor_mul(trash[:], cnt_sb[:], tsb[:])
                    nc.vector.reduce_sum(tp[:], trash[:],
                                         axis=mybir.AxisListType.X)
                    nc.vector.tensor_add(gold_sb[:], gold_sb[:], tp[:])
                nc.sync.dma_start(gold[:], gold_sb[:])

            if not _EN_EMIT:
                pieces = []

        # injection schedule (value: list of callables taking the current
        # scan-step anchor instruction)
        inject_at = {}
        if _EN_SCAN:
            for c in range(NEARLY, NCHUNK):
                k_need = CH_OFF[c] // PAIRW
                lead = 8 if c < 4 else 20
                inject_at.setdefault(max(2, k_need - lead), []).append(
                    lambda anchor, c=c: exp_chunk(c))
            if _EN_GOLD:
                for j in range(len(pieces)):
                    inject_at.setdefault(40 + 6 * j, []).append(
                        lambda anchor, j=j: emit_piece(j, anchor))
        else:
            for c in range(NEARLY, NCHUNK):
                exp_chunk(c)
            if _EN_GOLD:
                for j in range(len(pieces)):
                    emit_piece(j, None)

        if _EN_SCAN:
            # ---- renorm helper (paired F|B) -----------------------------
            pend = {}
            ren_i = [0]

            def renorm(state_ap, k):
                cs = ps.tile([1, PAIRW], F32, tag="cs", bufs=1, name=f"cs{k}")
                nc.tensor.matmul(cs[:], ones_col[:], state_ap,
                                 start=True, stop=True)
                j = ren_i[0]
                ren_i[0] += 1
                nc.scalar.copy(cspair_sb[0:1, j * PAIRW:(j + 1) * PAIRW], cs[:])
                rec = wk.tile([1, PAIRW], BF16, tag="rec", name=f"rec{k}")
                with nc.allow_low_precision(
                        reason="rescale factor; compensated via host log"):
                    nc.vector.reciprocal(rec[:], cs[:])
                bc = ps.tile([C, PAIRW], F32, tag="bc", name=f"bc{k}")
                nc.tensor.matmul(bc[:], ones_row[:], rec[:],
                                 start=True, stop=True)
                bsb = wk.tile([C, PAIRW], BF16, tag="bsb", name=f"bsb{k}")
                nc.scalar.copy(bsb[:], bc[:])
                s_apply = k + RDELAY
                es = wk.tile([C, PAIRW], BF16, tag="es", name=f"es{k}")
                nc.gpsimd.tensor_mul(es[:], ec_pair(s_apply), bsb[:])
                pend[s_apply] = es

            # ---- bidirectional paired scan ------------------------------
            a = ec_pair(0)        # [E_0 | E_511]
            for k in range(1, HALF):
                pp = ps.tile([C, PAIRW], F32, tag="pp", bufs=4, name=f"pp{k}")
                nc.tensor.matmul(pp[:, 0:BC], wf[:], a[:, 0:BC],
                                 start=True, stop=True)
                nc.tensor.matmul(pp[:, BC:PAIRW], wb_[:], a[:, BC:PAIRW],
                                 start=True, stop=True)
                ek = pend.pop(k, None)
                ek = ek[:] if ek is not None else ec_pair(k)
                a_new = wk.tile([C, PAIRW], BF16, tag="a", bufs=6, name=f"a{k}")
                tt_inst = nc.vector.tensor_tensor(a_new[:], pp[:], ek,
                                                  op=OP.mult)
                a = a_new[:]

                if k % RENORM == 0 and k + RDELAY < HALF:
                    renorm(a, k)
                for job in inject_at.get(k, []):
                    job(tt_inst)

            # ---- final renorm: keep the meeting product inside f32 ------
            csz_f = ps.tile([1, PAIRW], F32, tag="cs", bufs=1, name="cs_fin")
            nc.tensor.matmul(csz_f[:], ones_col[:], a, start=True, stop=True)
            jf = ren_i[0]
            nc.scalar.copy(cspair_sb[0:1, jf * PAIRW:(jf + 1) * PAIRW],
                           csz_f[:])
            rec_f = wk.tile([1, PAIRW], BF16, tag="rec", name="rec_fin")
            with nc.allow_low_precision(
                    reason="rescale factor; compensated via host log"):
                nc.vector.reciprocal(rec_f[:], csz_f[:])
            bc_f = ps.tile([C, PAIRW], F32, tag="bc", name="bc_fin")
            nc.tensor.matmul(bc_f[:], ones_row[:], rec_f[:],
                             start=True, stop=True)
            bsb_f = wk.tile([C, PAIRW], BF16, tag="bsb", name="bsb_fin")
            nc.scalar.copy(bsb_f[:], bc_f[:])
            a_fin = wk.tile([C, PAIRW], BF16, tag="a", bufs=6, name="a_fin")
            nc.vector.tensor_tensor(a_fin[:], a, bsb_f[:], op=OP.mult)
            a = a_fin[:]

            # ---- combine ------------------------------------------------
            pbf = ps.tile([C, BC], F32, tag="pp", bufs=4, name="pb_final")
            nc.tensor.matmul(pbf[:], wb_[:], a[:, BC:PAIRW],
                             start=True, stop=True)
            d = wk.tile([C, BC], BF16, tag="a", bufs=6, name="d_meet")
            nc.vector.tensor_tensor(d[:], pbf[:], a[:, 0:BC], op=OP.mult)
            pd = ps.tile([1, BC], F32, tag="cs", bufs=1, name="pd_final")
            nc.tensor.matmul(pd[:], ones_col[:], d[:], start=True, stop=True)
            pdsb = sb.tile([1, BC], F32, name="pdsb")
            nc.scalar.copy(pdsb[:], pd[:])
            nc.sync.dma_start(pdrow[:], pdsb[:])
            nc.sync.dma_start(cspair[:], cspair_sb[:])
        if _EN_GOLD:
            gold_finish()

    nc.compile()
    return nc


def _prep_inputs(emissions, tags, mask, transitions):
    em = np.asarray(emissions, dtype=np.float32)
    tg = np.asarray(tags).astype(np.int64)
    mk = np.asarray(mask).astype(np.float32)
    tr = np.ascontiguousarray(np.asarray(transitions, dtype=np.float32))

    a_f = np.exp(tr.astype(np.float64))
    afwd = a_f.astype(ml_dtypes.bfloat16)
    abwd = np.ascontiguousarray(a_f.T).astype(ml_dtypes.bfloat16)

    # paired free layout: pair-step k holds [E_k | E_{S-1-k}] in 64 cols
    s_all = np.arange(S, dtype=np.int64)
    pair_base = np.where(s_all < S // 2, s_all * PAIRW,
                         (S - 1 - s_all) * PAIRW + BC)   # [S]
    b_rows = np.arange(BC, dtype=np.int64)[:, None]      # [BC,1]
    sbcol = (pair_base[None, :] + b_rows).ravel()        # free idx for (b,s)

    in_maps = []
    for core in range(NCORES):
        b0 = core * BC
        ec = em[b0:b0 + BC]                              # [BC,S,C]
        ett = ec.transpose(2, 1, 0)                      # [C,S,BC]
        half = S // 2
        et = np.empty((C, half, PAIRW), dtype=np.float32)
        et[:, :, :BC] = ett[:, :half, :]                 # fwd slot: E_k
        et[:, :, BC:] = ett[:, :half - 1:-1, :]          # bwd slot: E_{S-1-k}
        et = np.ascontiguousarray(
            et.reshape(C, FREE)).astype(ml_dtypes.bfloat16)

        tgc = tg[b0:b0 + BC]                             # [BC,S]
        mkc = mk[b0:b0 + BC]

        hemit = np.zeros((C, FREE), dtype=ml_dtypes.bfloat16)
        hemit[tgc.ravel(), sbcol] = mkc.ravel()

        # masked pair-count histogram (index-only preprocessing; the
        # float gather-sum  sum T[i,j]*CNT[i,j]  runs on device)
        cnt = np.zeros((C, C), dtype=np.float64)
        np.add.at(cnt, (tgc[:, :-1].ravel(), tgc[:, 1:].ravel()),
                  mkc[:, 1:].ravel().astype(np.float64))
        cnt = cnt.astype(np.float32)

        in_maps.append({
            "et": et, "afwd": afwd, "abwd": abwd,
            "hemit": hemit, "cnt": cnt, "tsb": tr,
        })
    return in_maps


def kernel(emissions, tags, mask, transitions, _trace=False):
    global _NC_CACHE
    if _NC_CACHE is None:
        _NC_CACHE = _build_nc()
    nc = _NC_CACHE

    in_maps = _prep_inputs(emissions, tags, mask, transitions)
    res = run_bass_kernel_spmd(
        nc, in_maps, core_ids=list(range(NCORES)), trace=_trace,
    )
    partition = np.float64(0.0)
    gold = np.float64(0.0)
    for r in res.results:
        partition += np.log(np.asarray(r["pdrow"], dtype=np.float64)).sum()
        partition += np.log(np.asarray(r["cspair"], dtype=np.float64)).sum()
        gold += np.asarray(r["gold"], dtype=np.float64).sum()
    out = np.float32(partition - gold)
    if _trace:
        return out, res
    return out



# revision 22
# speedup vs baseline: 3.2698x; 3.2698x over previous
"""CRF negative-log-likelihood kernel for Trainium2 (8 NeuronCores).

Math: reference computes  partition - gold  where
  partition = sum_b log 1^T [prod_{t=511..1} (D_t A^T)] alpha_0,
  A = exp(T), D_t = diag(exp(e_t)), alpha_0 = exp(e_0);
  gold = sum emissions[b,s,tags]*m + sum T[tags[s],tags[s+1]]*m[:,1:].

Key idea (rank-1 segmentation): products of strictly positive matrices
contract to rank-1 at machine precision within ~10 steps.  Split the
511-step chain into NSEG=16 segments of L=32.  Each middle segment j is
M_j ~= a_j b_j^T / c_j with a_j = M_j 1 (forward vector chain),
b_j^T = 1^T M_j (backward chain), c_j = colsum(b_j); segments 0 and 15
contribute their single exact chain.  Per batch row
  log Z = sum_{p=0..14} log(b_{p+1}^T a_p) - sum_{j=1..14} log colsum(b_j)
          + L*(NSEG)*C0  (constant, see below).
All 30 chains are independent -> the scan is 32 wide steps of
[C,480] matmuls + elementwise multiplies instead of 256 narrow
latency-bound steps (validated: f64 segmentation error ~1e-13; a full
bf16 numpy model of this kernel matches the reference at rel 3.2e-6).

Numerical range: instead of data-driven renormalization, every E slice
is computed as exp(e - C0) with constant C0 ~ the mean per-step log
growth (~log(C e^1)).  Growth is then compensated every step and the
state magnitude performs a bounded random walk (measured |log| < 40 vs
bf16 range 88).  Host compensation is the exact constant C0 per
consumed slice; segment-internal factors cancel identically.

Device mapping (data-parallel over batch, 32 rows per core):
  * fwd state AF [C, 15*32]: block j = chain of segment j (j=0..14);
    bwd state WB holds E-premultiplied states for j=1..15.  Per step:
    2 matmuls (PE, bf16, full 128 contraction) into PSUM, 2 elementwise
    multiplies with the step's emission slice (DVE; GPSIMD cannot read
    PSUM - neuronxcc rejects it).  DVE is the saturated engine.
  * Emissions arrive raw bf16 in an l-pair-major host layout
    (col = POS[t%L]*512 + (t//L)*32 + b, POS = 0,31,1,30,...) so that
    every per-step slice for BOTH directions is contiguous in ONE
    shared array: fwd step s reads cols [POS[s]*512, +480), bwd reads
    [POS[31-s]*512+32, +480).  exp runs on ScalarE in storage order,
    which is exactly first-use order; DMA likewise.
  * Chain 0 starts from ones like the rest: its t=0 emission block is
    pre-divided by A^T 1 on device so D_0' A^T 1 = E_0 exactly.
  * gold emit: 128 accumulating PE matmuls hemit_chunk^T @ raw_chunk
    into one PSUM bank; sum(diag) = the masked gather-sum, extracted
    with an identity multiply + colsum.  gold trans: count matrix . T.
Host does integer/layout preprocessing and f64 log-sum postprocessing.
"""

import sys

for _p in ("/opt/trn_rl_repo",):
    if _p not in sys.path:
        sys.path.insert(0, _p)

import numpy as np
import ml_dtypes
from contextlib import ExitStack

from concourse import bass, tile, mybir, bacc
from concourse.bass_utils import run_bass_kernel_spmd
from concourse.tile_rust import add_dep_helper

NCORES = 8
B, S, C = 256, 512, 128
BC = B // NCORES          # 32 batch rows per core
NSEG = 16
L = S // NSEG             # 32 scan steps
NBLK = NSEG - 1           # 15 chains per direction
WF = NBLK * BC            # 480 state columns per direction
BLKW = NSEG * BC          # 512: one l-position across all segments
EFREE = S * BC            # 16384 emission columns (single copy)
C0 = 5.375                # constant per-step log-growth compensation

# storage position of l-value: l-pairs (0,31),(1,30),... in need order
POS = [0] * L
for _l in range(L):
    POS[_l] = 2 * _l if _l < L // 2 else 2 * (L - 1 - _l) + 1

# DMA/exp chunk boundaries in storage *positions* (each position = 512 cols);
# position m is first needed at scan step m//2.
CH_POS = [0, 1, 2] + list(range(4, 34, 2))
NCH = len(CH_POS) - 1
EXP_LEAD = 6              # inject exp of chunk k this many steps early

F32 = mybir.dt.float32
BF16 = mybir.dt.bfloat16
AF = mybir.ActivationFunctionType
OP = mybir.AluOpType

_NC_CACHE = None


def _fwd_off(s):
    return POS[s] * BLKW


def _bwd_off(s):
    # bwd mult at loop iter s consumes l = 31-s; block j=0 excluded
    return POS[L - 1 - s] * BLKW + BC


def _build_nc():
    nc = bacc.Bacc("TRN2", target_bir_lowering=False, debug=False)

    eraw_in = nc.dram_tensor("eraw", [C, EFREE], BF16, kind="ExternalInput").ap()
    afwd = nc.dram_tensor("afwd", [C, C], BF16, kind="ExternalInput").ap()
    abwd = nc.dram_tensor("abwd", [C, C], BF16, kind="ExternalInput").ap()
    hem_in = nc.dram_tensor("hem", [C, EFREE], BF16, kind="ExternalInput").ap()
    cnt_in = nc.dram_tensor("cnt", [C, C], F32, kind="ExternalInput").ap()
    bias0_in = nc.dram_tensor("bias0", [C, 1], F32, kind="ExternalInput").ap()
    tsb_in = nc.dram_tensor("tsb", [C, C], F32, kind="ExternalInput").ap()
    id_in = nc.dram_tensor("ident", [C, C], BF16, kind="ExternalInput").ap()

    # single combined output row: [dots | bcol | emit | trans]
    OUTW = 2 * WF + 2 * C
    outs_o = nc.dram_tensor("outs", [1, OUTW], F32, kind="ExternalOutput").ap()

    with tile.TileContext(nc) as tc, ExitStack() as ctx:
        sb = ctx.enter_context(tc.tile_pool(name="sb", bufs=1))
        wk = ctx.enter_context(tc.tile_pool(name="wk", bufs=4))
        ps = ctx.enter_context(tc.tile_pool(name="ps", bufs=2, space="PSUM"))

        # ---- weights first (tiny), then the emission stream -------------
        wf = sb.tile([C, C], BF16, name="wf")
        wb_ = sb.tile([C, C], BF16, name="wb")
        nc.gpsimd.dma_start(wf[:], afwd[:])
        nc.gpsimd.dma_start(wb_[:], abwd[:])

        bias0_sb = sb.tile([C, 1], F32, name="bias0_sb")
        nc.gpsimd.dma_start(bias0_sb[:], bias0_in[:])
        eraw_sb = sb.tile([C, EFREE], BF16, name="eraw_sb")
        em_dmas = []
        for k in range(NCH):
            o, e = CH_POS[k] * BLKW, CH_POS[k + 1] * BLKW
            em_dmas.append(nc.sync.dma_start(eraw_sb[:, o:e], eraw_in[:, o:e]))

        ones_col = sb.tile([C, 1], BF16, name="ones_col")
        ones_row = sb.tile([1, C], BF16, name="ones_row")
        nc.vector.memset(ones_col[:], 1.0)
        nc.vector.memset(ones_row[:], 1.0)

        # gold + tail-only inputs stream after the emission chunks
        hem_sb = sb.tile([C, EFREE], BF16, name="hem_sb")
        anchor = em_dmas[-1].ins
        qs = EFREE // 8
        for k in range(8):
            gd = nc.sync.dma_start(hem_sb[:, k * qs:(k + 1) * qs],
                                   hem_in[:, k * qs:(k + 1) * qs])
            add_dep_helper(gd.ins, anchor, reason="gold DMA after emissions")
        cnt_sb = sb.tile([C, C], F32, name="cnt_sb")
        tsb_sb = sb.tile([C, C], F32, name="tsb_sb")
        id_sb = sb.tile([C, C], BF16, name="id_sb")
        for gd in (nc.sync.dma_start(cnt_sb[:], cnt_in[:]),
                   nc.sync.dma_start(tsb_sb[:], tsb_in[:]),
                   nc.sync.dma_start(id_sb[:], id_in[:])):
            add_dep_helper(gd.ins, anchor, reason="tail DMA after emissions")

        # ---- exp chunks on ScalarE (with constant range bias) -----------
        E = sb.tile([C, EFREE], BF16, name="E")
        bias_c0 = sb.tile([C, 1], F32, name="bias_c0")
        nc.vector.memset(bias_c0[:], -C0)
        warm = sb.tile([C, 1], BF16, name="warm")
        nc.scalar.activation(warm[:], ones_col[:], AF.Exp)

        def exp_chunk(k):
            o, e = CH_POS[k] * BLKW, CH_POS[k + 1] * BLKW
            if k == 0:
                # chain-0 seed: E_0' = exp(e_0 - C0 - log(A^T 1)) so that
                # D_0' A^T ones = E_0 exactly (t=0 lives at cols 0:BC)
                nc.scalar.activation(E[:, 0:BC], eraw_sb[:, 0:BC], AF.Exp,
                                     bias=bias0_sb[:])
                nc.scalar.activation(E[:, BC:e], eraw_sb[:, BC:e], AF.Exp,
                                     bias=bias_c0[:])
                return
            nc.scalar.activation(E[:, o:e], eraw_sb[:, o:e], AF.Exp,
                                 bias=bias_c0[:])

        exp_chunk(0)

        # rowsum(wb) = colsum(wf), used by the bcol matmul at step 30
        aw = ps.tile([C, 1], F32, tag="cs", bufs=2, name="aw")
        nc.tensor.matmul(aw[:], wf[:], ones_col[:], start=True, stop=True)
        awb = sb.tile([C, 1], BF16, name="awb")
        nc.vector.tensor_copy(awb[:], aw[:])

        exp_chunk(1)

        # ---- scan -------------------------------------------------------
        af = wk.tile([C, WF], BF16, tag="af", bufs=3, name="af_init")
        nc.vector.memset(af[:], 1.0)

        emit_ps = ps.tile([C, C], F32, tag="emit", bufs=1, name="emit_ps")
        NEMIT = EFREE // C
        emit_n = [0]

        def emit_mm():
            g = emit_n[0]
            if g >= NEMIT:
                return
            emit_n[0] += 1
            nc.tensor.matmul(emit_ps[:], hem_sb[:, g * C:(g + 1) * C],
                             eraw_sb[:, g * C:(g + 1) * C],
                             start=(g == 0), stop=(g == NEMIT - 1))

        outs_sb = sb.tile([1, OUTW], F32, name="outs_sb")
        wbst = None  # bwd SBUF state; step 0 feeds the E slice directly
        for s in range(L):
            ppF = ps.tile([C, WF], F32, tag="ppF", bufs=2, name=f"ppF{s}")
            nc.tensor.matmul(ppF[:], wf[:], af[:], start=True, stop=True)
            brhs = wbst[:] if wbst is not None \
                else E[:, _bwd_off(0):_bwd_off(0) + WF]
            ppB = ps.tile([C, WF], F32, tag="ppB", bufs=2, name=f"ppB{s}")
            nc.tensor.matmul(ppB[:], wb_[:], brhs, start=True, stop=True)

            o = _fwd_off(s)
            af_new = wk.tile([C, WF], BF16, tag="af", bufs=3, name=f"af{s + 1}")
            nc.vector.tensor_tensor(af_new[:], ppF[:], E[:, o:o + WF],
                                    op=OP.mult)
            af = af_new

            if s < L - 1:
                o = _bwd_off(s + 1)
                wb_new = wk.tile([C, WF], BF16, tag="wbs", bufs=3,
                                 name=f"wbs{s + 1}")
                nc.vector.tensor_tensor(wb_new[:], ppB[:], E[:, o:o + WF],
                                        op=OP.mult)
                wbst = wb_new

            # inject exp chunks ahead of need (chunk k live at step
            # CH_POS[k]//2)
            for k in range(2, NCH):
                if s == max(0, CH_POS[k] // 2 - EXP_LEAD):
                    exp_chunk(k)
            # emit-gold matmuls fill PE idle slots once hemit streams in
            if s >= 4:
                for _ in range(6):
                    emit_mm()

            if s == 28:
                # trans gold (independent of the scan): colsum(cnt * T)
                tt = wk.tile([C, C], F32, tag="tt", bufs=1, name="tt")
                nc.gpsimd.tensor_tensor(tt[:], cnt_sb[:], tsb_sb[:],
                                        op=OP.mult)
                ttb = wk.tile([C, C], BF16, tag="ide", bufs=2, name="ttb")
                nc.gpsimd.tensor_copy(ttb[:], tt[:])
                tps = ps.tile([1, C], F32, tag="cs", bufs=2, name="trans_cs")
                nc.tensor.matmul(tps[:], ones_col[:], ttb[:],
                                 start=True, stop=True)
                nc.scalar.copy(outs_sb[0:1, 2 * WF + C:], tps[:])
            if s == 29:
                # emit gold: sum(diag(emit_ps)) via identity mask + colsum
                assert emit_n[0] >= NEMIT
                ide = wk.tile([C, C], BF16, tag="ide", bufs=2, name="ide")
                nc.vector.tensor_tensor(ide[:], emit_ps[:], id_sb[:],
                                        op=OP.mult)
                eps2 = ps.tile([1, C], F32, tag="cs", bufs=2, name="emit_cs")
                nc.tensor.matmul(eps2[:], ones_col[:], ide[:],
                                 start=True, stop=True)
                nc.scalar.copy(outs_sb[0:1, 2 * WF:2 * WF + C], eps2[:])
            if s == 30:
                # bcol = colsum(wb^T WB31) = rowsum(wb)^T WB31; rowsum(wb)
                # equals colsum(wf) = aw, so one [1,WF] matmul off the
                # SBUF state replaces a PSUM evacuation after step 31.
                csB = ps.tile([1, WF], F32, tag="cs", bufs=2, name="csB_fin")
                nc.tensor.matmul(csB[:], awb[:], wbst[:],
                                 start=True, stop=True)
                nc.scalar.copy(outs_sb[0:1, WF:2 * WF], csB[:])

        # ---- tail: boundary dots off the final states -------------------
        # block p of both states holds the segment pair (p+1, p)
        d = wk.tile([C, WF], BF16, tag="es", bufs=2, name="dmeet")
        nc.vector.tensor_tensor(d[:], ppB[:], af[:], op=OP.mult)
        dps = ps.tile([1, WF], F32, tag="cs", bufs=2, name="dots_ps")
        nc.tensor.matmul(dps[:], ones_col[:], d[:], start=True, stop=True)
        nc.scalar.copy(outs_sb[0:1, 0:WF], dps[:])

        nc.sync.dma_start(outs_o[:], outs_sb[:])

    nc.compile()
    return nc


def _prep_inputs(emissions, tags, mask, transitions):
    em = np.asarray(emissions, dtype=np.float32)
    tg = np.asarray(tags).astype(np.int64)
    mk = np.asarray(mask).astype(np.float32)
    tr = np.ascontiguousarray(np.asarray(transitions, dtype=np.float32))

    a_f = np.exp(tr.astype(np.float64))
    afwd = a_f.astype(ml_dtypes.bfloat16)
    abwd = np.ascontiguousarray(a_f.T).astype(ml_dtypes.bfloat16)
    ident = np.eye(C, dtype=ml_dtypes.bfloat16)
    # chain-0 seed bias: -C0 - log(colsum of bf16(A)) per tag
    aw_host = afwd.astype(np.float64).sum(axis=0)
    bias0 = (-C0 - np.log(aw_host)).astype(np.float32).reshape(C, 1)

    # t_idx[m, j] = j*L + O[m]: storage position m holds l-value O[m]
    O = np.empty(L, dtype=np.int64)
    for l in range(L):
        O[POS[l]] = l
    t_idx = np.arange(NSEG)[None, :] * L + O[:, None]    # [m, j]

    in_maps = []
    for core in range(NCORES):
        b0 = core * BC
        ecsb = em[b0:b0 + BC].transpose(2, 1, 0)         # [C, S, BC]
        eraw = np.ascontiguousarray(
            ecsb[:, t_idx, :].reshape(C, EFREE)).astype(ml_dtypes.bfloat16)

        tgc = tg[b0:b0 + BC]                             # [BC, S]
        mkc = mk[b0:b0 + BC]
        hfull = np.zeros((C, S, BC), dtype=np.float32)
        s_all = np.arange(S)
        b_all = np.arange(BC)
        bb, ss = np.meshgrid(b_all, s_all, indexing="ij")
        hfull[tgc.ravel(), ss.ravel(), bb.ravel()] = mkc.ravel()
        hem = np.ascontiguousarray(
            hfull[:, t_idx, :].reshape(C, EFREE)).astype(ml_dtypes.bfloat16)

        cntm = np.zeros((C, C), dtype=np.float64)
        np.add.at(cntm, (tgc[:, :-1].ravel(), tgc[:, 1:].ravel()),
                  mkc[:, 1:].ravel().astype(np.float64))

        in_maps.append({
            "eraw": eraw, "afwd": afwd, "abwd": abwd, "hem": hem,
            "cnt": cntm.astype(np.float32), "tsb": tr, "ident": ident,
            "bias0": bias0,
        })
    return in_maps


def kernel(emissions, tags, mask, transitions, _trace=False):
    global _NC_CACHE
    if _NC_CACHE is None:
        _NC_CACHE = _build_nc()
    nc = _NC_CACHE

    in_maps = _prep_inputs(emissions, tags, mask, transitions)
    res = run_bass_kernel_spmd(
        nc, in_maps, core_ids=list(range(NCORES)), trace=_trace,
    )
    partition = np.float64(0.0)
    gold = np.float64(0.0)
    for r in res.results:
        outs = np.asarray(r["outs"], dtype=np.float64).ravel()
        dots = outs[0:WF].reshape(NBLK, BC)
        bcol = outs[WF:2 * WF].reshape(NBLK, BC)
        partition += np.log(dots).sum()
        partition -= np.log(bcol[:NBLK - 1]).sum()
        partition += C0 * L * (NBLK + 1) * BC
        gold += outs[2 * WF:].sum()
    out = np.float32(partition - gold)
    if _trace:
        return out, res
    return out
